# revision 22
# baseline (speedup 1.0000x reference)
"""Trainium2 Bass kernel for BertSimSelfAttention (sparse_attention).

Problem (full): B=4, M=64, SEQ=256, DIM=1024, H=16, HD=64.
Effective batch rows R = B*SEQ = 1024, each row: m=64 tokens of dim=1024.
  hs  = transpose(hidden_states,(0,2,1,3)).reshape(R, 64, 1024)
  q/k/v = hs @ W{q,k,v}.T + b   (per token)
  per (row, head): scores = (q @ k.T)/8 * sim[row] + (-1e4)*(1-am[row,j])
  probs = softmax_j(scores);  ctx = probs @ v  -> out [R, 64, 1024]

Sharding: data-parallel over rows, 128 rows/core x 8 cores. The host
pre-transposes x, W (and sim per row), casts them to bf16, and folds
the 1/sqrt(hd) scale into Wq/bq.

Per-core design:
  - xT [d, t] and WT [d, o] in bf16, DMA'd directly into SBUF tiles.
  - Projections all-bf16 (1 cyc/row on PE): qT in [o, t] layout bf16
    (heads on partition strips by head parity), v natural [t, o] bf16.
    q bias added by ACT Identity at evacuation. k is evacuated
    STRAIGHT INTO block-diagonal stationary form: per (row, head-pair)
    a [128, 128] tile diag(k_even^T | k_odd^T) whose zero blocks are
    memset once and persist (two ping-pong kd buffers, one per tile
    parity). v bias accumulated as a K=1 bf16 matmul (skipped when
    bv == 0).
  - scores: 8 full-width matmuls per row (stationary = kd block, FWL
    eligible; moving = qT slice [128, 64]) -> S'[j, q] in one PSUM
    bank [128 = 2x64 j, 512 = 8 head-pairs x 64 q].
  - softmax, flash-style: t = S' * simT (DVE, sim broadcast via
    stride-0 AP; one batched sim DMA per tile); e = exp(t + maskcol)
    on ACT with the additive key mask as a per-partition bias column.
    The exp is written straight into block-diagonal stationary form
    ed = diag(e_even | e_odd) per head-pair (zero blocks persist,
    4 ping-pong buffers), in 4 [64, 256] calls so ctx of head-pairs
    0-3 can start while 4-7 still exponentiate.
  - ctx + denominators fused: 8 matmuls per row (stationary = ed
    block, moving = [v_h | 1] 65-column blocks, ones columns persist)
    -> PSUM [q, 0:64] = unnormalized ctx, [q, 64] = softmax denom.
    Final DVE pass multiplies by reciprocal_approx_fast(denom) while
    evacuating PSUM -> SBUF. No probs tensor ever materializes.
  - DMA queues: sync + scalar (HW DGE) carry weights/x/out; gpsimd
    (SW queue) only memsets, sim and v-block fills.
  - Emission software-pipelines tile i's projection groups with tile
    (i-1)'s attention rows so the PE stream stays dense.
"""

import sys

sys.path.insert(0, "/opt/trn_rl_repo")

import numpy as np
import concourse.bass as bass
import concourse.bacc as bacc
import concourse.mybir as mybir
import concourse.tile as tile

F32 = mybir.dt.float32
BF16 = mybir.dt.bfloat16
AF = mybir.ActivationFunctionType
ALU = mybir.AluOpType

N_CORES = 8
M = 64                    # tokens per row
DIM = 1024
H = 16
HD = 64
NEG = -10000.0


def build_core_kernel(nc, n_tiles=16, rows_per_tile=8, use_bv=True):
    """Emit the per-core program. tile = rows_per_tile rows (must be even)."""
    T_TILE = rows_per_tile * M        # tokens per tile (512 default)
    n_rows = n_tiles * rows_per_tile
    n_tok = n_rows * M
    SUB = T_TILE // 128               # 128-token subtiles per tile

    xt_d = nc.dram_tensor("xT", (DIM, n_tok), BF16, kind="ExternalInput")
    sim_d = nc.dram_tensor("simg", (n_rows, M, M), F32, kind="ExternalInput")
    am_d = nc.dram_tensor("am", (n_rows, M), F32, kind="ExternalInput")
    wq_d = nc.dram_tensor("WqT", (DIM, DIM), BF16, kind="ExternalInput")
    wk_d = nc.dram_tensor("WkT", (DIM, DIM), BF16, kind="ExternalInput")
    wv_d = nc.dram_tensor("WvT", (DIM, DIM), BF16, kind="ExternalInput")
    bq_d = nc.dram_tensor("bq", (DIM,), F32, kind="ExternalInput")
    bk_d = nc.dram_tensor("bk", (DIM,), F32, kind="ExternalInput")
    bv_d = nc.dram_tensor("bv", (DIM,), F32, kind="ExternalInput")
    id_d = nc.dram_tensor("ident", (128, 128), F32, kind="ExternalInput")
    out_d = nc.dram_tensor("out", (n_tok, DIM), F32, kind="ExternalOutput")

    with tile.TileContext(nc) as tc:
        with (
            tc.tile_pool(name="consts", bufs=1) as consts,
            tc.tile_pool(name="xtp", bufs=2) as xtp,
            tc.tile_pool(name="qkp", bufs=2) as qkp,
            tc.tile_pool(name="vp", bufs=2) as vp,
            tc.tile_pool(name="rowp", bufs=2) as rowp,
            tc.tile_pool(name="proj_ps", bufs=2, space="PSUM") as proj_ps,
            tc.tile_pool(name="att_ps", bufs=2, space="PSUM") as att_ps,
        ):
            # ---------------- tiny consts first ----------------
            ident = consts.tile([128, 128], F32)
            nc.sync.dma_start(ident[:], id_d[:])

            am_all = consts.tile([128, M], F32)
            if n_rows < 128:
                nc.gpsimd.memset(am_all[:], 1.0)
            nc.sync.dma_start(am_all[0:n_rows, :], am_d[:])

            bq_sb = consts.tile([128, 8], F32)
            bk_sb = consts.tile([128, 8], F32)
            nc.sync.dma_start(bq_sb[:], bq_d[:].rearrange("(o p) -> p o", p=128))
            nc.sync.dma_start(bk_sb[:], bk_d[:].rearrange("(o p) -> p o", p=128))

            if use_bv:
                # bv as a K=1 bf16 pair for psum-accumulate
                ones_f = consts.tile([1, 128], F32)
                nc.gpsimd.memset(ones_f[:], 1.0)
                ones_r = consts.tile([1, 128], BF16)
                nc.vector.tensor_copy(ones_r[:], ones_f[:])
                bv_row = consts.tile([1, DIM], F32)
                nc.sync.dma_start(bv_row[:],
                                  bv_d[:].rearrange("(a o) -> a o", a=1))
                bv_r = consts.tile([1, DIM], BF16)
                nc.vector.tensor_copy(bv_r[:], bv_row[:])

            # block-diagonal k stationaries: per (row, head-pair) a
            # [128, 128] block diag(k_even^T | k_odd^T). Zero blocks are
            # memset once and persist; the k-projection evacuation
            # rewrites only the diagonal blocks. Two buffers ping-pong
            # by tile parity.
            kd_bufs = []
            for i in range(2):
                kd = consts.tile([128, rows_per_tile * 8 * 128], BF16,
                                 name=f"kd{i}")
                nc.gpsimd.memset(kd[:], 0.0)
                kd_bufs.append(kd)

            # block-diagonal e stationaries (one row each), 4 ping-pong
            ed_bufs = []
            for i in range(4):
                ed = consts.tile([128, 8 * 128], BF16, name=f"ed{i}")
                nc.gpsimd.memset(ed[:], 0.0)
                ed_bufs.append(ed)

            # v-moving buffers: per head-pair hp a 65-col block
            # [128 = v_even | v_odd, 65 = hd | 1]; ones columns persist.
            v2bufs = []
            for i in range(4):
                v2b = consts.tile([128, 8 * 65], BF16, name=f"v2b{i}")
                nc.gpsimd.memset(v2b[:], 1.0)
                v2bufs.append(v2b)

            # mask bias columns: mcolT2[:, r] = -1e4*(1 - am[r, j]) on both
            # partition halves (exp-bias per key token j)
            mcolT2 = consts.tile([128, 128], F32)
            amt_ps = att_ps.tile([128, 512], F32, tag="att")
            nc.tensor.transpose(amt_ps[0:M, 0:128], am_all[:], ident[:])
            nc.vector.tensor_scalar(
                mcolT2[0:64, :], amt_ps[0:M, 0:128], -NEG, NEG,
                op0=ALU.mult, op1=ALU.add)
            nc.vector.tensor_scalar(
                mcolT2[64:128, :], amt_ps[0:M, 0:128], -NEG, NEG,
                op0=ALU.mult, op1=ALU.add)

            # ---------------- weights + x tiles: direct bf16 DMA ---------
            # sync + scalar are HW DGE queues and carry all bulk traffic;
            # gpsimd (slow SW queue) keeps only memsets/sim/v-fills. DMA
            # issues on the scalar queue are safe only when they can never
            # block (buffers 3-deep), else they'd stall ACT evacuations.
            qhw = [nc.sync, nc.scalar]

            def emit_xt(ti):
                t0 = ti * T_TILE
                xt = [xtp.tile([128, T_TILE], BF16, tag=f"xt{d}", bufs=3,
                               name=f"xt{d}_{ti}") for d in range(8)]
                for dch in range(8):
                    nc.sync.dma_start(
                        xt[dch][:],
                        xt_d[128 * dch:128 * dch + 128, t0:t0 + T_TILE])
                return xt

            def emit_sim(ti):
                # all 8 rows' simT for the tile in 2 DMAs
                r0 = ti * rows_per_tile
                simt8 = rowp.tile([128, T_TILE], F32, tag="sim8",
                                  name=f"sim8_{ti}")
                src = sim_d[r0:r0 + rows_per_tile, :, :].rearrange(
                    "r j q -> j r q")
                for st in (0, 64):
                    nc.gpsimd.dma_start(
                        simt8[st:st + 64, :]
                        .rearrange("j (r q) -> j r q", q=M),
                        src)
                return simt8

            # tile-0 x chunks interleaved with wq chunks on the two HW
            # queues so the first q-projection matmuls can start after
            # ~2 transfers; wk/wv follow while tile-0 q-projections run.
            xt0 = [xtp.tile([128, T_TILE], BF16, tag=f"xt{d}", bufs=3,
                            name=f"xt{d}_0") for d in range(8)]
            wts = {name: [consts.tile([128, DIM], BF16, tag=f"w{name}{d}",
                                      name=f"w{name}{d}") for d in range(8)]
                   for name in ("q", "k", "v")}
            for dch in range(8):
                qhw[dch % 2].dma_start(
                    wts["q"][dch][:], wq_d[128 * dch:128 * dch + 128, :])
                qhw[(dch + 1) % 2].dma_start(
                    xt0[dch][:], xt_d[128 * dch:128 * dch + 128, 0:T_TILE])
            for wi, (name, w_d) in enumerate(
                    (("k", wk_d), ("v", wv_d))):
                for dch in range(8):
                    qhw[(wi + dch) % 2].dma_start(
                        wts[name][dch][:],
                        w_d[128 * dch:128 * dch + 128, :])
            wqt, wkt, wvt = wts["q"], wts["k"], wts["v"]

            # ---------------- main loop over token tiles ----------------
            def make_proj(ti, xt):
                qt = [qkp.tile([128, T_TILE], BF16, tag=f"qt{o}",
                               name=f"qt{o}_{ti}") for o in range(8)]
                kd = kd_bufs[ti % 2]
                vts = [vp.tile([128, DIM], BF16, tag=f"v{s}",
                               name=f"v{s}_{ti}") for s in range(SUB)]
                groups = []

                def q_group(och):
                    ps = proj_ps.tile([128, T_TILE], F32, tag="proj",
                                      name=f"qkps{och}_{ti}")
                    for dch in range(8):
                        nc.tensor.matmul(
                            ps[:],
                            wqt[dch][:, 128 * och:128 * och + 128],
                            xt[dch][:],
                            start=(dch == 0), stop=(dch == 7),
                        )
                    nc.scalar.activation(
                        qt[och][:], ps[:], AF.Identity,
                        bias=bq_sb[:, och:och + 1], scale=1.0,
                    )

                def k_group(och):
                    # evacuate k straight into block-diagonal stationary
                    # form: kd[p<64, rr*1024 + och*128 + c] = k_even,
                    # kd[p>=64, ... + 64 + c] = k_odd; zero blocks persist
                    ps = proj_ps.tile([128, T_TILE], F32, tag="proj",
                                      name=f"kps{och}_{ti}")
                    for dch in range(8):
                        nc.tensor.matmul(
                            ps[:],
                            wkt[dch][:, 128 * och:128 * och + 128],
                            xt[dch][:],
                            start=(dch == 0), stop=(dch == 7),
                        )
                    kdv = kd[:].rearrange("p (rr blk) -> p rr blk", blk=1024)
                    for st in (0, 64):
                        nc.scalar.activation(
                            kdv[st:st + 64, :,
                                128 * och + st:128 * och + st + 64],
                            ps[st:st + 64, :]
                            .rearrange("p (rr c) -> p rr c", c=M),
                            AF.Identity,
                            bias=bk_sb[st:st + 64, och:och + 1], scale=1.0,
                        )

                def v_group(sub, oh):
                    vt = vts[sub]
                    ps = proj_ps.tile([128, 512], F32, tag="proj",
                                      name=f"vps{sub}{oh}_{ti}")
                    sl = slice(512 * oh, 512 * oh + 512)
                    for dch in range(8):
                        nc.tensor.matmul(
                            ps[:],
                            xt[dch][:, 128 * sub:128 * sub + 128],
                            wvt[dch][:, 512 * oh:512 * oh + 512],
                            start=(dch == 0), stop=(dch == 7) and not use_bv,
                        )
                    if use_bv:
                        nc.tensor.matmul(
                            ps[:], ones_r[:], bv_r[:, sl],
                            start=False, stop=True,
                        )
                    # DVE, not ACT: keeps the ACT FIFO short so q/k psum
                    # evacuations (which gate the PE) never queue behind it
                    nc.vector.tensor_copy(vt[:, sl], ps[:])

                for och in range(8):
                    groups.append(lambda och=och: q_group(och))
                for och in range(8):
                    groups.append(lambda och=och: k_group(och))
                for sub in range(SUB):
                    for oh in range(2):
                        groups.append(lambda sub=sub, oh=oh: v_group(sub, oh))
                return qt, kd, vts, groups

            def make_att_rows(ti, qt, kd, vts, simt8, la=1):
                rowstate = {}

                def att_row_a(rr):
                    r = ti * rows_per_tile + rr

                    # prefetch this row's v blocks one pipeline step early
                    rp = rr % 2
                    vt = vts[rr // 2]
                    v2b = v2bufs[rr % 4]
                    vsrc = (vt[64 * rp:64 * rp + 64, :]
                            .rearrange("p (b two c) -> p b two c",
                                       two=2, c=64))
                    v2bv = v2b[:].rearrange("p (b c) -> p b c", c=65)
                    for i, st in enumerate((0, 64)):
                        nc.sync.dma_start(
                            v2bv[st:st + 64, :, 0:64], vsrc[:, :, i, :])

                    # scores transposed: S'[j, q]; stationary = kd block
                    # (block-diagonal pair of heads), moving = qT slice
                    s_ps = att_ps.tile([128, 512], F32, tag="att",
                                       name=f"s_{r}")
                    tsl = slice(M * rr, M * rr + M)
                    for hp in range(8):
                        nc.tensor.matmul(
                            s_ps[:, 64 * hp:64 * hp + 64],
                            kd[:, 1024 * rr + 128 * hp:
                               1024 * rr + 128 * hp + 128],
                            qt[hp][:, tsl],
                            start=True, stop=True,
                        )

                    # t = S' * simT (DVE); ed = exp(t + maskcol) (ACT),
                    # written straight into block-diagonal stationary form,
                    # head-pairs 0-3 | 4-7 so ctx can start on the first
                    # half while the second is still on ACT
                    tt = rowp.tile([128, 512], F32, tag="tt", name=f"tt_{r}")
                    ed = ed_bufs[rr % 4]
                    edv = ed[:].rearrange("p (b c) -> p b c", c=128)
                    simr = simt8[:, M * rr:M * rr + M]
                    for half0 in (0, 256):
                        hsl = slice(half0, half0 + 256)
                        bsl = slice(half0 // 64, half0 // 64 + 4)
                        nc.vector.tensor_tensor(
                            tt[:, hsl].rearrange("p (a j) -> p a j", j=M),
                            s_ps[:, hsl].rearrange("p (a j) -> p a j", j=M),
                            simr.rearrange("p (a j) -> p a j", a=1)
                            .broadcast_to([128, 4, M]),
                            op=ALU.mult,
                        )
                        for st in (0, 64):
                            nc.scalar.activation(
                                edv[st:st + 64, bsl, st:st + 64],
                                tt[st:st + 64, hsl]
                                .rearrange("p (b c) -> p b c", c=M),
                                AF.Exp, bias=mcolT2[st:st + 64, r:r + 1])
                    rowstate[rr] = ed

                def att_row_b(rr):
                    r = ti * rows_per_tile + rr
                    ed = rowstate.pop(rr)
                    v2b = v2bufs[rr % 4]

                    # ctx' and denominators in one pass: stationary = ed
                    # block, moving = [v_h | 1] -> out[q, 0:64] = ctx',
                    # out[q, 64] = denom. Head-pairs 0-3 -> ctxa, 4-7 ->
                    # ctxb ([128,512] banks; only cols 0:260 used).
                    ctxa = att_ps.tile([128, 512], F32, tag="ctxa",
                                       name=f"ctxa_{r}")
                    ctxb = att_ps.tile([128, 512], F32, tag="ctxb",
                                       name=f"ctxb_{r}")
                    for hp in range(8):
                        dst = ctxa if hp < 4 else ctxb
                        col = 65 * (hp % 4)
                        nc.tensor.matmul(
                            dst[:, col:col + 65],
                            ed[:, 128 * hp:128 * hp + 128],
                            v2b[:, 65 * hp:65 * hp + 65],
                            start=True, stop=True,
                        )

                    # normalize while evacuating: out = ctx' * 1/denom
                    rcp = rowp.tile([128, 8], F32, tag="rcp", name=f"rcp_{r}")
                    osb = rowp.tile([128, 512], F32, tag="osb",
                                    name=f"osb_{r}")
                    for i, cx in enumerate((ctxa, ctxb)):
                        cxv = cx[:, 0:260].rearrange("p (b c) -> p b c", c=65)
                        nc.vector.reciprocal_approx_fast(
                            out=rcp[:, 4 * i:4 * i + 4]
                            .rearrange("p (b c) -> p b c", c=1),
                            in_=cxv[:, :, 64:65])
                        nc.vector.tensor_tensor(
                            osb[:, 256 * i:256 * i + 256]
                            .rearrange("p (b c) -> p b c", c=64),
                            cxv[:, :, 0:64],
                            rcp[:, 4 * i:4 * i + 4]
                            .rearrange("p (b o) -> p b o", o=1)
                            .broadcast_to([128, 4, 64]),
                            op=ALU.mult)

                    # out[64r + q, 64h + hd]; strip par holds heads 2hp+par
                    ov = out_d[M * r:M * r + M, :].rearrange(
                        "q (hp two hd) -> q hp two hd", two=2, hd=64)
                    for par in range(2):
                        nc.sync.dma_start(
                            ov[:, :, par, :],
                            osb[64 * par:64 * par + 64, :]
                            .rearrange("q (hp hd) -> q hp hd", hd=64),
                        )

                units = [lambda rr=rr: att_row_a(rr) for rr in range(la)]
                for rr in range(la, rows_per_tile):
                    units.append(lambda rr=rr: att_row_a(rr))
                    units.append(lambda rr=rr: att_row_b(rr - la))
                for rr in range(rows_per_tile - la, rows_per_tile):
                    units.append(lambda rr=rr: att_row_b(rr))
                return units

            prev_rows = []
            for ti in range(n_tiles):
                xt = xt0 if ti == 0 else emit_xt(ti)
                simt8 = emit_sim(ti)
                qt, kd, vts, groups = make_proj(ti, xt)
                ri = 0
                for gi, g in enumerate(groups):
                    g()
                    while (ri < len(prev_rows)
                           and (gi + 1) * len(prev_rows) // len(groups) > ri):
                        prev_rows[ri]()
                        ri += 1
                while ri < len(prev_rows):
                    prev_rows[ri]()
                    ri += 1
                prev_rows = make_att_rows(
                    ti, qt, kd, vts, simt8,
                    la=(3 if ti == n_tiles - 1 else 1))
            for row in prev_rows:
                row()

    return dict(out=out_d)


def _prepare_shards(hidden_states, attention_mask, sim_graph, Wq, bq, Wk, bk, Wv, bv,
                    n_cores=N_CORES):
    from ml_dtypes import bfloat16
    b, m, seq, dim = hidden_states.shape
    R = b * seq
    hs = np.transpose(np.asarray(hidden_states), (0, 2, 1, 3)).reshape(R, m, dim)
    am = np.ascontiguousarray(
        np.transpose(np.asarray(attention_mask), (0, 2, 1)).reshape(R, m),
        dtype=np.float32)
    sim = np.ascontiguousarray(
        np.transpose(np.asarray(sim_graph), (0, 2, 1)), dtype=np.float32)
    ident = np.eye(128, dtype=np.float32)
    WqT = np.ascontiguousarray((np.asarray(Wq).T * 0.125).astype(bfloat16))
    WkT = np.ascontiguousarray(np.asarray(Wk).T.astype(bfloat16))
    WvT = np.ascontiguousarray(np.asarray(Wv).T.astype(bfloat16))
    rows_per_core = R // n_cores
    in_maps = []
    for c in range(n_cores):
        r0 = c * rows_per_core
        xT = np.ascontiguousarray(
            hs[r0:r0 + rows_per_core].reshape(rows_per_core * m, dim)
            .T.astype(bfloat16))
        in_maps.append(dict(
            xT=xT,
            simg=sim[r0:r0 + rows_per_core],
            am=am[r0:r0 + rows_per_core],
            WqT=WqT, WkT=WkT, WvT=WvT,
            bq=np.ascontiguousarray(np.asarray(bq) * 0.125, np.float32),
            bk=np.ascontiguousarray(bk, np.float32),
            bv=np.ascontiguousarray(bv, np.float32),
            ident=ident,
        ))
    return in_maps


_CACHE = {}


def _get_compiled(use_bv=True):
    key = ("nc", use_bv)
    if key not in _CACHE:
        nc = bacc.Bacc("TRN2", target_bir_lowering=False, debug=False)
        build_core_kernel(nc, use_bv=use_bv)
        nc.compile()
        _CACHE[key] = nc
    return _CACHE[key]


LAST_EXEC_NS = [None]


def kernel(hidden_states, attention_mask, sim_graph, Wq, bq, Wk, bk, Wv, bv,
           b=4, m=64, seq=256, dim=1024, **_):
    from concourse import bass2jax

    use_bv = bool(np.any(np.asarray(bv)))
    nc = _get_compiled(use_bv=use_bv)
    in_maps = _prepare_shards(hidden_states, attention_mask, sim_graph,
                              Wq, bq, Wk, bk, Wv, bv)
    res = bass2jax.run_bass_via_pjrt(nc, in_maps, n_cores=N_CORES)
    R = int(b) * int(seq)
    out = np.concatenate([res[c]["out"] for c in range(N_CORES)], axis=0)
    return out.reshape(R, int(m), int(dim))


# revision 27
# speedup vs baseline: 1.0310x; 1.0310x over previous
"""Trainium2 Bass kernel for BertSimSelfAttention (sparse_attention).

Problem (full): B=4, M=64, SEQ=256, DIM=1024, H=16, HD=64.
Effective batch rows R = B*SEQ = 1024, each row: m=64 tokens of dim=1024.
  hs  = transpose(hidden_states,(0,2,1,3)).reshape(R, 64, 1024)
  q/k/v = hs @ W{q,k,v}.T + b   (per token)
  per (row, head): scores = (q @ k.T)/8 * sim[row] + (-1e4)*(1-am[row,j])
  probs = softmax_j(scores);  ctx = probs @ v  -> out [R, 64, 1024]

Sharding: data-parallel over rows, 128 rows/core x 8 cores. The host
pre-transposes x, W (and sim per row), casts them to bf16, and folds
the 1/sqrt(hd) scale into Wq/bq.

Per-core design:
  - xT [d, t] and WT [d, o] in bf16, DMA'd directly into SBUF tiles.
  - Projections all-bf16 (1 cyc/row on PE): qT in [o, t] layout bf16
    (heads on partition strips by head parity), v natural [t, o] bf16.
    q bias added by ACT Identity at evacuation. k is evacuated
    STRAIGHT INTO block-diagonal stationary form: per (row, head-pair)
    a [128, 128] tile diag(k_even^T | k_odd^T) whose zero blocks are
    memset once and persist (two ping-pong kd buffers, one per tile
    parity). v bias accumulated as a K=1 bf16 matmul (skipped when
    bv == 0).
  - scores: 8 full-width matmuls per row (stationary = kd block, FWL
    eligible; moving = qT slice [128, 64]) -> S'[j, q] in one PSUM
    bank [128 = 2x64 j, 512 = 8 head-pairs x 64 q].
  - softmax, flash-style: t = S' * simT (DVE, sim broadcast via
    stride-0 AP; one batched sim DMA per tile); e = exp(t + maskcol)
    on ACT with the additive key mask as a per-partition bias column.
    The exp is written straight into block-diagonal stationary form
    ed = diag(e_even | e_odd) per head-pair (zero blocks persist,
    4 ping-pong buffers), in 4 [64, 256] calls so ctx of head-pairs
    0-3 can start while 4-7 still exponentiate.
  - ctx + denominators fused: 8 matmuls per row (stationary = ed
    block, moving = [v_h | 1] 65-column blocks, ones columns persist)
    -> PSUM [q, 0:64] = unnormalized ctx, [q, 64] = softmax denom.
    Final DVE pass multiplies by reciprocal_approx_fast(denom) while
    evacuating PSUM -> SBUF. No probs tensor ever materializes.
  - DMA queues: sync + scalar (HW DGE) carry weights/x/out; gpsimd
    (SW queue) only memsets, sim and v-block fills.
  - Emission software-pipelines tile i's projection groups with tile
    (i-1)'s attention rows so the PE stream stays dense.
"""

import sys

sys.path.insert(0, "/opt/trn_rl_repo")

import numpy as np
import concourse.bass as bass
import concourse.bacc as bacc
import concourse.mybir as mybir
import concourse.tile as tile

F32 = mybir.dt.float32
BF16 = mybir.dt.bfloat16
AF = mybir.ActivationFunctionType
ALU = mybir.AluOpType

N_CORES = 8
M = 64                    # tokens per row
DIM = 1024
H = 16
HD = 64
NEG = -10000.0


def build_core_kernel(nc, n_tiles=16, rows_per_tile=8, use_bv=True):
    """Emit the per-core program. tile = rows_per_tile rows (must be even)."""
    T_TILE = rows_per_tile * M        # tokens per tile (512 default)
    n_rows = n_tiles * rows_per_tile
    n_tok = n_rows * M
    SUB = T_TILE // 128               # 128-token subtiles per tile

    xt_d = nc.dram_tensor("xT", (DIM, n_tok), BF16, kind="ExternalInput")
    sim_d = nc.dram_tensor("simg", (n_rows, M, M), F32, kind="ExternalInput")
    am_d = nc.dram_tensor("am", (n_rows, M), F32, kind="ExternalInput")
    wq_d = nc.dram_tensor("WqT", (DIM, DIM), BF16, kind="ExternalInput")
    wk_d = nc.dram_tensor("WkT", (DIM, DIM), BF16, kind="ExternalInput")
    wv_d = nc.dram_tensor("WvT", (DIM, DIM), BF16, kind="ExternalInput")
    bq_d = nc.dram_tensor("bq", (DIM,), F32, kind="ExternalInput")
    bk_d = nc.dram_tensor("bk", (DIM,), F32, kind="ExternalInput")
    bv_d = nc.dram_tensor("bv", (DIM,), F32, kind="ExternalInput")
    id_d = nc.dram_tensor("ident", (128, 128), F32, kind="ExternalInput")
    out_d = nc.dram_tensor("out", (n_tok, DIM), F32, kind="ExternalOutput")

    with tile.TileContext(nc) as tc:
        with (
            tc.tile_pool(name="consts", bufs=1) as consts,
            tc.tile_pool(name="xtp", bufs=2) as xtp,
            tc.tile_pool(name="qkp", bufs=2) as qkp,
            tc.tile_pool(name="vp", bufs=2) as vp,
            tc.tile_pool(name="rowp", bufs=2) as rowp,
            tc.tile_pool(name="proj_ps", bufs=2, space="PSUM") as proj_ps,
            tc.tile_pool(name="att_ps", bufs=2, space="PSUM") as att_ps,
        ):
            # ---------------- tiny consts first ----------------
            ident = consts.tile([128, 128], F32)
            nc.sync.dma_start(ident[:], id_d[:])

            am_all = consts.tile([128, M], F32)
            if n_rows < 128:
                nc.gpsimd.memset(am_all[:], 1.0)
            nc.sync.dma_start(am_all[0:n_rows, :], am_d[:])

            bq_sb = consts.tile([128, 8], F32)
            bk_sb = consts.tile([128, 8], F32)
            nc.sync.dma_start(bq_sb[:], bq_d[:].rearrange("(o p) -> p o", p=128))
            nc.sync.dma_start(bk_sb[:], bk_d[:].rearrange("(o p) -> p o", p=128))

            if use_bv:
                # bv as a K=1 bf16 pair for psum-accumulate
                ones_f = consts.tile([1, 128], F32)
                nc.gpsimd.memset(ones_f[:], 1.0)
                ones_r = consts.tile([1, 128], BF16)
                nc.vector.tensor_copy(ones_r[:], ones_f[:])
                bv_row = consts.tile([1, DIM], F32)
                nc.sync.dma_start(bv_row[:],
                                  bv_d[:].rearrange("(a o) -> a o", a=1))
                bv_r = consts.tile([1, DIM], BF16)
                nc.vector.tensor_copy(bv_r[:], bv_row[:])

            # block-diagonal k stationaries: per (row, head-pair) a
            # [128, 128] block diag(k_even^T | k_odd^T). Zero blocks are
            # memset once and persist; the k-projection evacuation
            # rewrites only the diagonal blocks. Two buffers ping-pong
            # by tile parity.
            kd_bufs = []
            for i in range(2):
                kd = consts.tile([128, rows_per_tile * 8 * 128], BF16,
                                 name=f"kd{i}")
                nc.gpsimd.memset(kd[:], 0.0)
                kd_bufs.append(kd)

            # block-diagonal e stationaries (one row each), 4 ping-pong
            ed_bufs = []
            for i in range(4):
                ed = consts.tile([128, 8 * 128], BF16, name=f"ed{i}")
                nc.gpsimd.memset(ed[:], 0.0)
                ed_bufs.append(ed)

            # v-moving buffers: per head-pair hp a 65-col block
            # [128 = v_even | v_odd, 65 = hd | 1]; ones columns persist.
            v2bufs = []
            for i in range(4):
                v2b = consts.tile([128, 8 * 65], BF16, name=f"v2b{i}")
                nc.gpsimd.memset(v2b[:], 1.0)
                v2bufs.append(v2b)

            # mask bias columns: mcolT2[:, r] = -1e4*(1 - am[r, j]) on both
            # partition halves (exp-bias per key token j)
            mcolT2 = consts.tile([128, 128], F32)
            amt_ps = att_ps.tile([128, 512], F32, tag="att")
            nc.tensor.transpose(amt_ps[0:M, 0:128], am_all[:], ident[:])
            nc.vector.tensor_scalar(
                mcolT2[0:64, :], amt_ps[0:M, 0:128], -NEG, NEG,
                op0=ALU.mult, op1=ALU.add)
            nc.vector.tensor_scalar(
                mcolT2[64:128, :], amt_ps[0:M, 0:128], -NEG, NEG,
                op0=ALU.mult, op1=ALU.add)

            # ---------------- weights + x tiles: direct bf16 DMA ---------
            # sync + scalar are HW DGE queues and carry all bulk traffic;
            # gpsimd (slow SW queue) keeps only memsets/sim/v-fills. DMA
            # issues on the scalar queue are safe only when they can never
            # block (buffers 3-deep), else they'd stall ACT evacuations.
            qhw = [nc.sync, nc.scalar]

            def emit_xt(ti):
                t0 = ti * T_TILE
                xt = [xtp.tile([128, T_TILE], BF16, tag=f"xt{d}", bufs=3,
                               name=f"xt{d}_{ti}") for d in range(8)]
                for dch in range(8):
                    nc.sync.dma_start(
                        xt[dch][:],
                        xt_d[128 * dch:128 * dch + 128, t0:t0 + T_TILE])
                return xt

            def emit_sim(ti):
                # all 8 rows' simT for the tile in 2 DMAs
                r0 = ti * rows_per_tile
                simt8 = rowp.tile([128, T_TILE], F32, tag="sim8",
                                  name=f"sim8_{ti}")
                src = sim_d[r0:r0 + rows_per_tile, :, :].rearrange(
                    "r j q -> j r q")
                for st in (0, 64):
                    nc.gpsimd.dma_start(
                        simt8[st:st + 64, :]
                        .rearrange("j (r q) -> j r q", q=M),
                        src)
                return simt8

            # tile-0 x chunks interleaved with wq chunks on the two HW
            # queues so the first q-projection matmuls can start after
            # ~2 transfers; wk/wv follow while tile-0 q-projections run.
            xt0 = [xtp.tile([128, T_TILE], BF16, tag=f"xt{d}", bufs=3,
                            name=f"xt{d}_0") for d in range(8)]
            wts = {name: [consts.tile([128, DIM], BF16, tag=f"w{name}{d}",
                                      name=f"w{name}{d}") for d in range(8)]
                   for name in ("q", "k", "v")}
            for dch in range(8):
                qhw[dch % 2].dma_start(
                    wts["q"][dch][:], wq_d[128 * dch:128 * dch + 128, :])
                qhw[(dch + 1) % 2].dma_start(
                    xt0[dch][:], xt_d[128 * dch:128 * dch + 128, 0:T_TILE])
            for wi, (name, w_d) in enumerate(
                    (("k", wk_d), ("v", wv_d))):
                for dch in range(8):
                    qhw[(wi + dch) % 2].dma_start(
                        wts[name][dch][:],
                        w_d[128 * dch:128 * dch + 128, :])
            wqt, wkt, wvt = wts["q"], wts["k"], wts["v"]

            # ---------------- main loop over token tiles ----------------
            def make_proj(ti, xt):
                qt = [qkp.tile([128, T_TILE], BF16, tag=f"qt{o}",
                               name=f"qt{o}_{ti}") for o in range(8)]
                kd = kd_bufs[ti % 2]
                vts = [vp.tile([128, DIM], BF16, tag=f"v{s}",
                               name=f"v{s}_{ti}") for s in range(SUB)]
                groups = []

                def q_group(och):
                    ps = proj_ps.tile([128, T_TILE], F32, tag="proj",
                                      name=f"qkps{och}_{ti}")
                    for dch in range(8):
                        nc.tensor.matmul(
                            ps[:],
                            wqt[dch][:, 128 * och:128 * och + 128],
                            xt[dch][:],
                            start=(dch == 0), stop=(dch == 7),
                        )
                    nc.scalar.activation(
                        qt[och][:], ps[:], AF.Identity,
                        bias=bq_sb[:, och:och + 1], scale=1.0,
                    )

                def k_group(och):
                    # evacuate k straight into block-diagonal stationary
                    # form: kd[p<64, rr*1024 + och*128 + c] = k_even,
                    # kd[p>=64, ... + 64 + c] = k_odd; zero blocks persist
                    ps = proj_ps.tile([128, T_TILE], F32, tag="proj",
                                      name=f"kps{och}_{ti}")
                    for dch in range(8):
                        nc.tensor.matmul(
                            ps[:],
                            wkt[dch][:, 128 * och:128 * och + 128],
                            xt[dch][:],
                            start=(dch == 0), stop=(dch == 7),
                        )
                    kdv = kd[:].rearrange("p (rr blk) -> p rr blk", blk=1024)
                    for st in (0, 64):
                        nc.scalar.activation(
                            kdv[st:st + 64, :,
                                128 * och + st:128 * och + st + 64],
                            ps[st:st + 64, :]
                            .rearrange("p (rr c) -> p rr c", c=M),
                            AF.Identity,
                            bias=bk_sb[st:st + 64, och:och + 1], scale=1.0,
                        )

                def v_group(sub, oh):
                    vt = vts[sub]
                    ps = proj_ps.tile([128, 512], F32, tag="proj",
                                      name=f"vps{sub}{oh}_{ti}")
                    sl = slice(512 * oh, 512 * oh + 512)
                    for dch in range(8):
                        nc.tensor.matmul(
                            ps[:],
                            xt[dch][:, 128 * sub:128 * sub + 128],
                            wvt[dch][:, 512 * oh:512 * oh + 512],
                            start=(dch == 0), stop=(dch == 7) and not use_bv,
                        )
                    if use_bv:
                        nc.tensor.matmul(
                            ps[:], ones_r[:], bv_r[:, sl],
                            start=False, stop=True,
                        )
                    # DVE, not ACT: keeps the ACT FIFO short so q/k psum
                    # evacuations (which gate the PE) never queue behind it
                    nc.vector.tensor_copy(vt[:, sl], ps[:])

                for och in range(8):
                    groups.append(lambda och=och: q_group(och))
                for och in range(8):
                    groups.append(lambda och=och: k_group(och))
                for sub in range(SUB):
                    for oh in range(2):
                        groups.append(lambda sub=sub, oh=oh: v_group(sub, oh))
                return qt, kd, vts, groups

            def make_att_rows(ti, qt, kd, vts, simt8, la=1):
                rowstate = {}

                def att_row_a(rr):
                    r = ti * rows_per_tile + rr

                    # prefetch this row's v blocks one pipeline step early
                    rp = rr % 2
                    vt = vts[rr // 2]
                    v2b = v2bufs[rr % 4]
                    vsrc = (vt[64 * rp:64 * rp + 64, :]
                            .rearrange("p (b two c) -> p b two c",
                                       two=2, c=64))
                    v2bv = v2b[:].rearrange("p (b c) -> p b c", c=65)
                    for i, st in enumerate((0, 64)):
                        nc.sync.dma_start(
                            v2bv[st:st + 64, :, 0:64], vsrc[:, :, i, :])

                    # scores transposed: S'[j, q]; stationary = kd block
                    # (block-diagonal pair of heads), moving = qT slice
                    s_ps = att_ps.tile([128, 512], F32, tag="att",
                                       name=f"s_{r}")
                    tsl = slice(M * rr, M * rr + M)
                    for hp in range(8):
                        nc.tensor.matmul(
                            s_ps[:, 64 * hp:64 * hp + 64],
                            kd[:, 1024 * rr + 128 * hp:
                               1024 * rr + 128 * hp + 128],
                            qt[hp][:, tsl],
                            start=True, stop=True,
                        )

                    # t = S' * simT (DVE); ed = exp(t + maskcol) (ACT),
                    # written straight into block-diagonal stationary form,
                    # head-pairs 0-3 | 4-7 so ctx can start on the first
                    # half while the second is still on ACT
                    tt = rowp.tile([128, 512], F32, tag="tt", name=f"tt_{r}")
                    ed = ed_bufs[rr % 4]
                    edv = ed[:].rearrange("p (b c) -> p b c", c=128)
                    simr = simt8[:, M * rr:M * rr + M]
                    nc.vector.tensor_tensor(
                        tt[:].rearrange("p (a j) -> p a j", j=M),
                        s_ps[:].rearrange("p (a j) -> p a j", j=M),
                        simr.rearrange("p (a j) -> p a j", a=1)
                        .broadcast_to([128, 8, M]),
                        op=ALU.mult,
                    )
                    for st in (0, 64):
                        nc.scalar.activation(
                            edv[st:st + 64, :, st:st + 64],
                            tt[st:st + 64, :]
                            .rearrange("p (b c) -> p b c", c=M),
                            AF.Exp, bias=mcolT2[st:st + 64, r:r + 1])
                    rowstate[rr] = ed

                def att_row_b(rr):
                    r = ti * rows_per_tile + rr
                    ed = rowstate.pop(rr)
                    v2b = v2bufs[rr % 4]

                    # ctx' and denominators in one pass: stationary = ed
                    # block, moving = [v_h | 1] -> out[q, 0:64] = ctx',
                    # out[q, 64] = denom. Head-pairs 0-3 -> ctxa, 4-7 ->
                    # ctxb ([128,512] banks; only cols 0:260 used).
                    ctxa = att_ps.tile([128, 512], F32, tag="ctxa",
                                       name=f"ctxa_{r}")
                    ctxb = att_ps.tile([128, 512], F32, tag="ctxb",
                                       name=f"ctxb_{r}")
                    for hp in range(8):
                        dst = ctxa if hp < 4 else ctxb
                        col = 65 * (hp % 4)
                        nc.tensor.matmul(
                            dst[:, col:col + 65],
                            ed[:, 128 * hp:128 * hp + 128],
                            v2b[:, 65 * hp:65 * hp + 65],
                            start=True, stop=True,
                        )

                    # normalize while evacuating: out = ctx' * 1/denom
                    rcp = rowp.tile([128, 8], F32, tag="rcp", name=f"rcp_{r}")
                    osb = rowp.tile([128, 512], F32, tag="osb",
                                    name=f"osb_{r}")
                    for i, cx in enumerate((ctxa, ctxb)):
                        cxv = cx[:, 0:260].rearrange("p (b c) -> p b c", c=65)
                        nc.vector.reciprocal_approx_fast(
                            out=rcp[:, 4 * i:4 * i + 4]
                            .rearrange("p (b c) -> p b c", c=1),
                            in_=cxv[:, :, 64:65])
                        nc.vector.tensor_tensor(
                            osb[:, 256 * i:256 * i + 256]
                            .rearrange("p (b c) -> p b c", c=64),
                            cxv[:, :, 0:64],
                            rcp[:, 4 * i:4 * i + 4]
                            .rearrange("p (b o) -> p b o", o=1)
                            .broadcast_to([128, 4, 64]),
                            op=ALU.mult)

                    # out[64r + q, 64h + hd]; strip par holds heads 2hp+par
                    ov = out_d[M * r:M * r + M, :].rearrange(
                        "q (hp two hd) -> q hp two hd", two=2, hd=64)
                    for par in range(2):
                        nc.sync.dma_start(
                            ov[:, :, par, :],
                            osb[64 * par:64 * par + 64, :]
                            .rearrange("q (hp hd) -> q hp hd", hd=64),
                        )

                units = [lambda rr=rr: att_row_a(rr) for rr in range(la)]
                for rr in range(la, rows_per_tile):
                    units.append(lambda rr=rr: att_row_a(rr))
                    units.append(lambda rr=rr: att_row_b(rr - la))
                for rr in range(rows_per_tile - la, rows_per_tile):
                    units.append(lambda rr=rr: att_row_b(rr))
                return units

            prev_rows = []
            for ti in range(n_tiles):
                xt = xt0 if ti == 0 else emit_xt(ti)
                simt8 = emit_sim(ti)
                qt, kd, vts, groups = make_proj(ti, xt)
                ri = 0
                for gi, g in enumerate(groups):
                    g()
                    while (ri < len(prev_rows)
                           and (gi + 1) * len(prev_rows) // len(groups) > ri):
                        prev_rows[ri]()
                        ri += 1
                while ri < len(prev_rows):
                    prev_rows[ri]()
                    ri += 1
                prev_rows = make_att_rows(
                    ti, qt, kd, vts, simt8,
                    la=(3 if ti == n_tiles - 1 else 1))
            for row in prev_rows:
                row()

    return dict(out=out_d)


def _prepare_shards(hidden_states, attention_mask, sim_graph, Wq, bq, Wk, bk, Wv, bv,
                    n_cores=N_CORES):
    from ml_dtypes import bfloat16
    b, m, seq, dim = hidden_states.shape
    R = b * seq
    hs = np.transpose(np.asarray(hidden_states), (0, 2, 1, 3)).reshape(R, m, dim)
    am = np.ascontiguousarray(
        np.transpose(np.asarray(attention_mask), (0, 2, 1)).reshape(R, m),
        dtype=np.float32)
    sim = np.ascontiguousarray(
        np.transpose(np.asarray(sim_graph), (0, 2, 1)), dtype=np.float32)
    ident = np.eye(128, dtype=np.float32)
    WqT = np.ascontiguousarray((np.asarray(Wq).T * 0.125).astype(bfloat16))
    WkT = np.ascontiguousarray(np.asarray(Wk).T.astype(bfloat16))
    WvT = np.ascontiguousarray(np.asarray(Wv).T.astype(bfloat16))
    rows_per_core = R // n_cores
    in_maps = []
    for c in range(n_cores):
        r0 = c * rows_per_core
        xT = np.ascontiguousarray(
            hs[r0:r0 + rows_per_core].reshape(rows_per_core * m, dim)
            .T.astype(bfloat16))
        in_maps.append(dict(
            xT=xT,
            simg=sim[r0:r0 + rows_per_core],
            am=am[r0:r0 + rows_per_core],
            WqT=WqT, WkT=WkT, WvT=WvT,
            bq=np.ascontiguousarray(np.asarray(bq) * 0.125, np.float32),
            bk=np.ascontiguousarray(bk, np.float32),
            bv=np.ascontiguousarray(bv, np.float32),
            ident=ident,
        ))
    return in_maps


_CACHE = {}


def _get_compiled(use_bv=True):
    key = ("nc", use_bv)
    if key not in _CACHE:
        nc = bacc.Bacc("TRN2", target_bir_lowering=False, debug=False)
        build_core_kernel(nc, use_bv=use_bv)
        nc.compile()
        _CACHE[key] = nc
    return _CACHE[key]


LAST_EXEC_NS = [None]


def kernel(hidden_states, attention_mask, sim_graph, Wq, bq, Wk, bk, Wv, bv,
           b=4, m=64, seq=256, dim=1024, **_):
    from concourse import bass2jax

    use_bv = bool(np.any(np.asarray(bv)))
    nc = _get_compiled(use_bv=use_bv)
    in_maps = _prepare_shards(hidden_states, attention_mask, sim_graph,
                              Wq, bq, Wk, bk, Wv, bv)
    res = bass2jax.run_bass_via_pjrt(nc, in_maps, n_cores=N_CORES)
    R = int(b) * int(seq)
    out = np.concatenate([res[c]["out"] for c in range(N_CORES)], axis=0)
    return out.reshape(R, int(m), int(dim))


# revision 30
# speedup vs baseline: 1.0512x; 1.0195x over previous
"""Trainium2 Bass kernel for BertSimSelfAttention (sparse_attention).

Problem (full): B=4, M=64, SEQ=256, DIM=1024, H=16, HD=64.
Effective batch rows R = B*SEQ = 1024, each row: m=64 tokens of dim=1024.
  hs  = transpose(hidden_states,(0,2,1,3)).reshape(R, 64, 1024)
  q/k/v = hs @ W{q,k,v}.T + b   (per token)
  per (row, head): scores = (q @ k.T)/8 * sim[row] + (-1e4)*(1-am[row,j])
  probs = softmax_j(scores);  ctx = probs @ v  -> out [R, 64, 1024]

Sharding: data-parallel over rows, 128 rows/core x 8 cores. The host
pre-transposes x, W (and sim per row), casts them to bf16, and folds
the 1/sqrt(hd) scale into Wq/bq.

Per-core design:
  - xT [d, t] and WT [d, o] in bf16, DMA'd directly into SBUF tiles.
  - Projections all-bf16 (1 cyc/row on PE): qT in [o, t] layout bf16
    (heads on partition strips by head parity), v natural [t, o] bf16.
    q bias added by ACT Identity at evacuation. k is evacuated
    STRAIGHT INTO block-diagonal stationary form: per (row, head-pair)
    a [128, 128] tile diag(k_even^T | k_odd^T) whose zero blocks are
    memset once and persist (two ping-pong kd buffers, one per tile
    parity). v bias accumulated as a K=1 bf16 matmul (skipped when
    bv == 0).
  - scores: 8 full-width matmuls per row (stationary = kd block, FWL
    eligible; moving = qT slice [128, 64]) -> S'[j, q] in one PSUM
    bank [128 = 2x64 j, 512 = 8 head-pairs x 64 q].
  - softmax, flash-style: t = S' * simT (DVE, sim broadcast via
    stride-0 AP; one batched sim DMA per tile); e = exp(t + maskcol)
    on ACT with the additive key mask as a per-partition bias column.
    The exp is written straight into block-diagonal stationary form
    ed = diag(e_even | e_odd) per head-pair (zero blocks persist,
    4 ping-pong buffers), in 4 [64, 256] calls so ctx of head-pairs
    0-3 can start while 4-7 still exponentiate.
  - ctx + denominators fused: 8 matmuls per row (stationary = ed
    block, moving = [v_h | 1] 65-column blocks, ones columns persist)
    -> PSUM [q, 0:64] = unnormalized ctx, [q, 64] = softmax denom.
    Final DVE pass multiplies by reciprocal_approx_fast(denom) while
    evacuating PSUM -> SBUF. No probs tensor ever materializes.
  - DMA queues: sync + scalar (HW DGE) carry weights/x/out; gpsimd
    (SW queue) only memsets, sim and v-block fills.
  - Emission software-pipelines tile i's projection groups with tile
    (i-1)'s attention rows so the PE stream stays dense.
"""

import sys

sys.path.insert(0, "/opt/trn_rl_repo")

import numpy as np
import concourse.bass as bass
import concourse.bacc as bacc
import concourse.mybir as mybir
import concourse.tile as tile

F32 = mybir.dt.float32
BF16 = mybir.dt.bfloat16
AF = mybir.ActivationFunctionType
ALU = mybir.AluOpType

N_CORES = 8
M = 64                    # tokens per row
DIM = 1024
H = 16
HD = 64
NEG = -10000.0


def build_core_kernel(nc, n_tiles=16, rows_per_tile=8, use_bv=True):
    """Emit the per-core program. tile = rows_per_tile rows (must be even)."""
    T_TILE = rows_per_tile * M        # tokens per tile (512 default)
    n_rows = n_tiles * rows_per_tile
    n_tok = n_rows * M
    SUB = T_TILE // 128               # 128-token subtiles per tile

    xt_d = nc.dram_tensor("xT", (DIM, n_tok), BF16, kind="ExternalInput")
    sim_d = nc.dram_tensor("simg", (n_rows, M, M), F32, kind="ExternalInput")
    am_d = nc.dram_tensor("am", (n_rows, M), F32, kind="ExternalInput")
    wq_d = nc.dram_tensor("WqT", (DIM, DIM), BF16, kind="ExternalInput")
    wk_d = nc.dram_tensor("WkT", (DIM, DIM), BF16, kind="ExternalInput")
    wv_d = nc.dram_tensor("WvT", (DIM, DIM), BF16, kind="ExternalInput")
    bq_d = nc.dram_tensor("bq", (DIM,), F32, kind="ExternalInput")
    bk_d = nc.dram_tensor("bk", (DIM,), F32, kind="ExternalInput")
    bv_d = nc.dram_tensor("bv", (DIM,), F32, kind="ExternalInput")
    id_d = nc.dram_tensor("ident", (128, 128), F32, kind="ExternalInput")
    out_d = nc.dram_tensor("out", (n_tok, DIM), F32, kind="ExternalOutput")

    with tile.TileContext(nc) as tc:
        with (
            tc.tile_pool(name="consts", bufs=1) as consts,
            tc.tile_pool(name="xtp", bufs=2) as xtp,
            tc.tile_pool(name="qkp", bufs=2) as qkp,
            tc.tile_pool(name="vp", bufs=2) as vp,
            tc.tile_pool(name="rowp", bufs=2) as rowp,
            tc.tile_pool(name="proj_ps", bufs=3, space="PSUM") as proj_ps,
            tc.tile_pool(name="att_ps", bufs=2, space="PSUM") as att_ps,
        ):
            # ---------------- tiny consts first ----------------
            ident = consts.tile([128, 128], F32)
            nc.sync.dma_start(ident[:], id_d[:])

            am_all = consts.tile([128, M], F32)
            if n_rows < 128:
                nc.vector.memset(am_all[:], 1.0)
            nc.sync.dma_start(am_all[0:n_rows, :], am_d[:])

            bq_sb = consts.tile([128, 8], F32)
            bk_sb = consts.tile([128, 8], F32)
            nc.sync.dma_start(bq_sb[:], bq_d[:].rearrange("(o p) -> p o", p=128))
            nc.sync.dma_start(bk_sb[:], bk_d[:].rearrange("(o p) -> p o", p=128))

            if use_bv:
                # bv as a K=1 bf16 pair for psum-accumulate
                ones_f = consts.tile([1, 128], F32)
                nc.vector.memset(ones_f[:], 1.0)
                ones_r = consts.tile([1, 128], BF16)
                nc.vector.tensor_copy(ones_r[:], ones_f[:])
                bv_row = consts.tile([1, DIM], F32)
                nc.sync.dma_start(bv_row[:],
                                  bv_d[:].rearrange("(a o) -> a o", a=1))
                bv_r = consts.tile([1, DIM], BF16)
                nc.vector.tensor_copy(bv_r[:], bv_row[:])

            # block-diagonal k stationaries: per (row, head-pair) a
            # [128, 128] block diag(k_even^T | k_odd^T). Zero blocks are
            # memset once and persist; the k-projection evacuation
            # rewrites only the diagonal blocks. Two buffers ping-pong
            # by tile parity.
            kd_bufs = []
            for i in range(2):
                kd = consts.tile([128, rows_per_tile * 8 * 128], BF16,
                                 name=f"kd{i}")
                nc.vector.memset(kd[:], 0.0)
                kd_bufs.append(kd)

            # block-diagonal e stationaries (one row each), 4 ping-pong
            ed_bufs = []
            for i in range(4):
                ed = consts.tile([128, 8 * 128], BF16, name=f"ed{i}")
                nc.vector.memset(ed[:], 0.0)
                ed_bufs.append(ed)

            # v-moving buffers: per head-pair hp a 65-col block
            # [128 = v_even | v_odd, 65 = hd | 1]; ones columns persist.
            v2bufs = []
            for i in range(4):
                v2b = consts.tile([128, 8 * 65], BF16, name=f"v2b{i}")
                nc.vector.memset(v2b[:], 1.0)
                v2bufs.append(v2b)

            # mask bias columns: mcolT2[:, r] = -1e4*(1 - am[r, j]) on both
            # partition halves (exp-bias per key token j)
            mcolT2 = consts.tile([128, 128], F32)
            amt_ps = att_ps.tile([128, 512], F32, tag="att")
            nc.tensor.transpose(amt_ps[0:M, 0:128], am_all[:], ident[:])
            nc.vector.tensor_scalar(
                mcolT2[0:64, :], amt_ps[0:M, 0:128], -NEG, NEG,
                op0=ALU.mult, op1=ALU.add)
            nc.vector.tensor_scalar(
                mcolT2[64:128, :], amt_ps[0:M, 0:128], -NEG, NEG,
                op0=ALU.mult, op1=ALU.add)

            # ---------------- weights + x tiles: direct bf16 DMA ---------
            # sync + scalar are HW DGE queues and carry all bulk traffic;
            # gpsimd (slow SW queue) keeps only memsets/sim/v-fills. DMA
            # issues on the scalar queue are safe only when they can never
            # block (buffers 3-deep), else they'd stall ACT evacuations.
            qhw = [nc.sync, nc.scalar]

            def emit_xt(ti):
                t0 = ti * T_TILE
                xt = [xtp.tile([128, T_TILE], BF16, tag=f"xt{d}", bufs=3,
                               name=f"xt{d}_{ti}") for d in range(8)]
                for dch in range(8):
                    nc.sync.dma_start(
                        xt[dch][:],
                        xt_d[128 * dch:128 * dch + 128, t0:t0 + T_TILE])
                return xt

            def emit_sim(ti):
                # all 8 rows' simT for the tile in 2 DMAs
                r0 = ti * rows_per_tile
                simt8 = rowp.tile([128, T_TILE], F32, tag="sim8",
                                  name=f"sim8_{ti}")
                src = sim_d[r0:r0 + rows_per_tile, :, :].rearrange(
                    "r j q -> j r q")
                for st in (0, 64):
                    nc.gpsimd.dma_start(
                        simt8[st:st + 64, :]
                        .rearrange("j (r q) -> j r q", q=M),
                        src)
                return simt8

            # tile-0 x chunks interleaved with wq chunks on the two HW
            # queues so the first q-projection matmuls can start after
            # ~2 transfers; wk/wv follow while tile-0 q-projections run.
            xt0 = [xtp.tile([128, T_TILE], BF16, tag=f"xt{d}", bufs=3,
                            name=f"xt{d}_0") for d in range(8)]
            wts = {name: [consts.tile([128, DIM], BF16, tag=f"w{name}{d}",
                                      name=f"w{name}{d}") for d in range(8)]
                   for name in ("q", "k", "v")}
            q3 = [nc.sync, nc.scalar, nc.gpsimd]
            for dch in range(8):
                q3[(2 * dch) % 3].dma_start(
                    wts["q"][dch][:], wq_d[128 * dch:128 * dch + 128, :])
                q3[(2 * dch + 1) % 3].dma_start(
                    xt0[dch][:], xt_d[128 * dch:128 * dch + 128, 0:T_TILE])
            for wi, (name, w_d) in enumerate(
                    (("k", wk_d), ("v", wv_d))):
                for dch in range(8):
                    q3[(wi + dch) % 3].dma_start(
                        wts[name][dch][:],
                        w_d[128 * dch:128 * dch + 128, :])
            wqt, wkt, wvt = wts["q"], wts["k"], wts["v"]

            # ---------------- main loop over token tiles ----------------
            def make_proj(ti, xt):
                qt = [qkp.tile([128, T_TILE], BF16, tag=f"qt{o}",
                               name=f"qt{o}_{ti}") for o in range(8)]
                kd = kd_bufs[ti % 2]
                vts = [vp.tile([128, DIM], BF16, tag=f"v{s}",
                               name=f"v{s}_{ti}") for s in range(SUB)]
                groups = []

                def q_group(och):
                    ps = proj_ps.tile([128, T_TILE], F32, tag="proj",
                                      name=f"qkps{och}_{ti}")
                    for dch in range(8):
                        nc.tensor.matmul(
                            ps[:],
                            wqt[dch][:, 128 * och:128 * och + 128],
                            xt[dch][:],
                            start=(dch == 0), stop=(dch == 7),
                        )
                    nc.scalar.activation(
                        qt[och][:], ps[:], AF.Identity,
                        bias=bq_sb[:, och:och + 1], scale=1.0,
                    )

                def k_group(och):
                    # evacuate k straight into block-diagonal stationary
                    # form: kd[p<64, rr*1024 + och*128 + c] = k_even,
                    # kd[p>=64, ... + 64 + c] = k_odd; zero blocks persist
                    ps = proj_ps.tile([128, T_TILE], F32, tag="proj",
                                      name=f"kps{och}_{ti}")
                    for dch in range(8):
                        nc.tensor.matmul(
                            ps[:],
                            wkt[dch][:, 128 * och:128 * och + 128],
                            xt[dch][:],
                            start=(dch == 0), stop=(dch == 7),
                        )
                    kdv = kd[:].rearrange("p (rr blk) -> p rr blk", blk=1024)
                    for st in (0, 64):
                        nc.scalar.activation(
                            kdv[st:st + 64, :,
                                128 * och + st:128 * och + st + 64],
                            ps[st:st + 64, :]
                            .rearrange("p (rr c) -> p rr c", c=M),
                            AF.Identity,
                            bias=bk_sb[st:st + 64, och:och + 1], scale=1.0,
                        )

                def v_group(sub, oh):
                    vt = vts[sub]
                    ps = proj_ps.tile([128, 512], F32, tag="proj",
                                      name=f"vps{sub}{oh}_{ti}")
                    sl = slice(512 * oh, 512 * oh + 512)
                    for dch in range(8):
                        nc.tensor.matmul(
                            ps[:],
                            xt[dch][:, 128 * sub:128 * sub + 128],
                            wvt[dch][:, 512 * oh:512 * oh + 512],
                            start=(dch == 0), stop=(dch == 7) and not use_bv,
                        )
                    if use_bv:
                        nc.tensor.matmul(
                            ps[:], ones_r[:], bv_r[:, sl],
                            start=False, stop=True,
                        )
                    # DVE, not ACT: keeps the ACT FIFO short so q/k psum
                    # evacuations (which gate the PE) never queue behind it
                    nc.vector.tensor_copy(vt[:, sl], ps[:])

                for och in range(8):
                    groups.append(lambda och=och: q_group(och))
                for och in range(8):
                    groups.append(lambda och=och: k_group(och))
                for sub in range(SUB):
                    for oh in range(2):
                        groups.append(lambda sub=sub, oh=oh: v_group(sub, oh))
                return qt, kd, vts, groups

            def make_att_rows(ti, qt, kd, vts, simt8, la=1):
                rowstate = {}

                def att_row_a(rr):
                    r = ti * rows_per_tile + rr

                    # prefetch this row's v blocks one pipeline step early
                    rp = rr % 2
                    vt = vts[rr // 2]
                    v2b = v2bufs[rr % 4]
                    vsrc = (vt[64 * rp:64 * rp + 64, :]
                            .rearrange("p (b two c) -> p b two c",
                                       two=2, c=64))
                    v2bv = v2b[:].rearrange("p (b c) -> p b c", c=65)
                    for i, st in enumerate((0, 64)):
                        nc.sync.dma_start(
                            v2bv[st:st + 64, :, 0:64], vsrc[:, :, i, :])

                    # scores transposed: S'[j, q]; stationary = kd block
                    # (block-diagonal pair of heads), moving = qT slice
                    s_ps = att_ps.tile([128, 512], F32, tag="att",
                                       name=f"s_{r}")
                    tsl = slice(M * rr, M * rr + M)
                    for hp in range(8):
                        nc.tensor.matmul(
                            s_ps[:, 64 * hp:64 * hp + 64],
                            kd[:, 1024 * rr + 128 * hp:
                               1024 * rr + 128 * hp + 128],
                            qt[hp][:, tsl],
                            start=True, stop=True,
                        )

                    # t = S' * simT (DVE); ed = exp(t + maskcol) (ACT),
                    # written straight into block-diagonal stationary form,
                    # head-pairs 0-3 | 4-7 so ctx can start on the first
                    # half while the second is still on ACT
                    tt = rowp.tile([128, 512], F32, tag="tt", name=f"tt_{r}")
                    ed = ed_bufs[rr % 4]
                    edv = ed[:].rearrange("p (b c) -> p b c", c=128)
                    simr = simt8[:, M * rr:M * rr + M]
                    nc.vector.tensor_tensor(
                        tt[:].rearrange("p (a j) -> p a j", j=M),
                        s_ps[:].rearrange("p (a j) -> p a j", j=M),
                        simr.rearrange("p (a j) -> p a j", a=1)
                        .broadcast_to([128, 8, M]),
                        op=ALU.mult,
                    )
                    for st in (0, 64):
                        nc.scalar.activation(
                            edv[st:st + 64, :, st:st + 64],
                            tt[st:st + 64, :]
                            .rearrange("p (b c) -> p b c", c=M),
                            AF.Exp, bias=mcolT2[st:st + 64, r:r + 1])
                    rowstate[rr] = ed

                def att_row_b(rr):
                    r = ti * rows_per_tile + rr
                    ed = rowstate.pop(rr)
                    v2b = v2bufs[rr % 4]

                    # ctx' and denominators in one pass: stationary = ed
                    # block, moving = [v_h | 1] -> out[q, 0:64] = ctx',
                    # out[q, 64] = denom. Head-pairs 0-3 -> ctxa, 4-7 ->
                    # ctxb ([128,512] banks; only cols 0:260 used).
                    ctxa = att_ps.tile([128, 512], F32, tag="ctxa",
                                       name=f"ctxa_{r}")
                    ctxb = att_ps.tile([128, 512], F32, tag="ctxb",
                                       name=f"ctxb_{r}", bufs=1)
                    for hp in range(8):
                        dst = ctxa if hp < 4 else ctxb
                        col = 65 * (hp % 4)
                        nc.tensor.matmul(
                            dst[:, col:col + 65],
                            ed[:, 128 * hp:128 * hp + 128],
                            v2b[:, 65 * hp:65 * hp + 65],
                            start=True, stop=True,
                        )

                    # normalize while evacuating: out = ctx' * 1/denom
                    rcp = rowp.tile([128, 8], F32, tag="rcp", name=f"rcp_{r}")
                    osb = rowp.tile([128, 512], F32, tag="osb",
                                    name=f"osb_{r}")
                    for i, cx in enumerate((ctxa, ctxb)):
                        cxv = cx[:, 0:260].rearrange("p (b c) -> p b c", c=65)
                        nc.vector.reciprocal_approx_fast(
                            out=rcp[:, 4 * i:4 * i + 4]
                            .rearrange("p (b c) -> p b c", c=1),
                            in_=cxv[:, :, 64:65])
                        nc.vector.tensor_tensor(
                            osb[:, 256 * i:256 * i + 256]
                            .rearrange("p (b c) -> p b c", c=64),
                            cxv[:, :, 0:64],
                            rcp[:, 4 * i:4 * i + 4]
                            .rearrange("p (b o) -> p b o", o=1)
                            .broadcast_to([128, 4, 64]),
                            op=ALU.mult)

                    # out[64r + q, 64h + hd]; strip par holds heads 2hp+par
                    ov = out_d[M * r:M * r + M, :].rearrange(
                        "q (hp two hd) -> q hp two hd", two=2, hd=64)
                    for par in range(2):
                        nc.sync.dma_start(
                            ov[:, :, par, :],
                            osb[64 * par:64 * par + 64, :]
                            .rearrange("q (hp hd) -> q hp hd", hd=64),
                        )

                units = [lambda rr=rr: att_row_a(rr) for rr in range(la)]
                for rr in range(la, rows_per_tile):
                    units.append(lambda rr=rr: att_row_a(rr))
                    units.append(lambda rr=rr: att_row_b(rr - la))
                for rr in range(rows_per_tile - la, rows_per_tile):
                    units.append(lambda rr=rr: att_row_b(rr))
                return units

            prev_rows = []
            for ti in range(n_tiles):
                xt = xt0 if ti == 0 else emit_xt(ti)
                simt8 = emit_sim(ti)
                qt, kd, vts, groups = make_proj(ti, xt)
                ri = 0
                for gi, g in enumerate(groups):
                    g()
                    while (ri < len(prev_rows)
                           and (gi + 1) * len(prev_rows) // len(groups) > ri):
                        prev_rows[ri]()
                        ri += 1
                while ri < len(prev_rows):
                    prev_rows[ri]()
                    ri += 1
                prev_rows = make_att_rows(
                    ti, qt, kd, vts, simt8,
                    la=(3 if ti == n_tiles - 1 else 1))
            for row in prev_rows:
                row()

    return dict(out=out_d)


def _prepare_shards(hidden_states, attention_mask, sim_graph, Wq, bq, Wk, bk, Wv, bv,
                    n_cores=N_CORES):
    from ml_dtypes import bfloat16
    b, m, seq, dim = hidden_states.shape
    R = b * seq
    hs = np.transpose(np.asarray(hidden_states), (0, 2, 1, 3)).reshape(R, m, dim)
    am = np.ascontiguousarray(
        np.transpose(np.asarray(attention_mask), (0, 2, 1)).reshape(R, m),
        dtype=np.float32)
    sim = np.ascontiguousarray(
        np.transpose(np.asarray(sim_graph), (0, 2, 1)), dtype=np.float32)
    ident = np.eye(128, dtype=np.float32)
    WqT = np.ascontiguousarray((np.asarray(Wq).T * 0.125).astype(bfloat16))
    WkT = np.ascontiguousarray(np.asarray(Wk).T.astype(bfloat16))
    WvT = np.ascontiguousarray(np.asarray(Wv).T.astype(bfloat16))
    rows_per_core = R // n_cores
    in_maps = []
    for c in range(n_cores):
        r0 = c * rows_per_core
        xT = np.ascontiguousarray(
            hs[r0:r0 + rows_per_core].reshape(rows_per_core * m, dim)
            .T.astype(bfloat16))
        in_maps.append(dict(
            xT=xT,
            simg=sim[r0:r0 + rows_per_core],
            am=am[r0:r0 + rows_per_core],
            WqT=WqT, WkT=WkT, WvT=WvT,
            bq=np.ascontiguousarray(np.asarray(bq) * 0.125, np.float32),
            bk=np.ascontiguousarray(bk, np.float32),
            bv=np.ascontiguousarray(bv, np.float32),
            ident=ident,
        ))
    return in_maps


_CACHE = {}


def _get_compiled(use_bv=True):
    key = ("nc", use_bv)
    if key not in _CACHE:
        nc = bacc.Bacc("TRN2", target_bir_lowering=False, debug=False)
        build_core_kernel(nc, use_bv=use_bv)
        nc.compile()
        _CACHE[key] = nc
    return _CACHE[key]


LAST_EXEC_NS = [None]


def kernel(hidden_states, attention_mask, sim_graph, Wq, bq, Wk, bk, Wv, bv,
           b=4, m=64, seq=256, dim=1024, **_):
    from concourse import bass2jax

    use_bv = bool(np.any(np.asarray(bv)))
    nc = _get_compiled(use_bv=use_bv)
    in_maps = _prepare_shards(hidden_states, attention_mask, sim_graph,
                              Wq, bq, Wk, bk, Wv, bv)
    res = bass2jax.run_bass_via_pjrt(nc, in_maps, n_cores=N_CORES)
    R = int(b) * int(seq)
    out = np.concatenate([res[c]["out"] for c in range(N_CORES)], axis=0)
    return out.reshape(R, int(m), int(dim))


# revision 31
# speedup vs baseline: 1.0515x; 1.0003x over previous
"""Trainium2 Bass kernel for BertSimSelfAttention (sparse_attention).

Problem (full): B=4, M=64, SEQ=256, DIM=1024, H=16, HD=64.
Effective batch rows R = B*SEQ = 1024, each row: m=64 tokens of dim=1024.
  hs  = transpose(hidden_states,(0,2,1,3)).reshape(R, 64, 1024)
  q/k/v = hs @ W{q,k,v}.T + b   (per token)
  per (row, head): scores = (q @ k.T)/8 * sim[row] + (-1e4)*(1-am[row,j])
  probs = softmax_j(scores);  ctx = probs @ v  -> out [R, 64, 1024]

Sharding: data-parallel over rows, 128 rows/core x 8 cores. The host
pre-transposes x, W (and sim per row), casts them to bf16, and folds
the 1/sqrt(hd) scale into Wq/bq.

Per-core design:
  - xT [d, t] and WT [d, o] in bf16, DMA'd directly into SBUF tiles.
  - Projections all-bf16 (1 cyc/row on PE): qT in [o, t] layout bf16
    (heads on partition strips by head parity), v natural [t, o] bf16.
    q bias added by ACT Identity at evacuation. k is evacuated
    STRAIGHT INTO block-diagonal stationary form: per (row, head-pair)
    a [128, 128] tile diag(k_even^T | k_odd^T) whose zero blocks are
    memset once and persist (two ping-pong kd buffers, one per tile
    parity). v bias accumulated as a K=1 bf16 matmul (skipped when
    bv == 0).
  - scores: 8 full-width matmuls per row (stationary = kd block, FWL
    eligible; moving = qT slice [128, 64]) -> S'[j, q] in one PSUM
    bank [128 = 2x64 j, 512 = 8 head-pairs x 64 q].
  - softmax, flash-style: t = S' * simT (DVE, sim broadcast via
    stride-0 AP; one batched sim DMA per tile); e = exp(t + maskcol)
    on ACT with the additive key mask as a per-partition bias column.
    The exp is written straight into block-diagonal stationary form
    ed = diag(e_even | e_odd) per head-pair (zero blocks persist,
    4 ping-pong buffers), in 4 [64, 256] calls so ctx of head-pairs
    0-3 can start while 4-7 still exponentiate.
  - ctx + denominators fused: 8 matmuls per row (stationary = ed
    block, moving = [v_h | 1] 65-column blocks, ones columns persist)
    -> PSUM [q, 0:64] = unnormalized ctx, [q, 64] = softmax denom.
    Final DVE pass multiplies by reciprocal_approx_fast(denom) while
    evacuating PSUM -> SBUF. No probs tensor ever materializes.
  - DMA queues: sync + scalar (HW DGE) carry weights/x/out; gpsimd
    (SW queue) only memsets, sim and v-block fills.
  - Emission software-pipelines tile i's projection groups with tile
    (i-1)'s attention rows so the PE stream stays dense.
"""

import sys

sys.path.insert(0, "/opt/trn_rl_repo")

import numpy as np
import concourse.bass as bass
import concourse.bacc as bacc
import concourse.mybir as mybir
import concourse.tile as tile

F32 = mybir.dt.float32
BF16 = mybir.dt.bfloat16
AF = mybir.ActivationFunctionType
ALU = mybir.AluOpType

N_CORES = 8
M = 64                    # tokens per row
DIM = 1024
H = 16
HD = 64
NEG = -10000.0


def build_core_kernel(nc, n_tiles=16, rows_per_tile=8, use_bv=True):
    """Emit the per-core program. tile = rows_per_tile rows (must be even)."""
    T_TILE = rows_per_tile * M        # tokens per tile (512 default)
    n_rows = n_tiles * rows_per_tile
    n_tok = n_rows * M
    SUB = T_TILE // 128               # 128-token subtiles per tile

    xt_d = nc.dram_tensor("xT", (DIM, n_tok), BF16, kind="ExternalInput")
    sim_d = nc.dram_tensor("simg", (n_rows, M, M), F32, kind="ExternalInput")
    am_d = nc.dram_tensor("am", (n_rows, M), F32, kind="ExternalInput")
    wq_d = nc.dram_tensor("WqT", (DIM, DIM), BF16, kind="ExternalInput")
    wk_d = nc.dram_tensor("WkT", (DIM, DIM), BF16, kind="ExternalInput")
    wv_d = nc.dram_tensor("WvT", (DIM, DIM), BF16, kind="ExternalInput")
    bq_d = nc.dram_tensor("bq", (DIM,), F32, kind="ExternalInput")
    bk_d = nc.dram_tensor("bk", (DIM,), F32, kind="ExternalInput")
    bv_d = nc.dram_tensor("bv", (DIM,), F32, kind="ExternalInput")
    id_d = nc.dram_tensor("ident", (128, 128), F32, kind="ExternalInput")
    out_d = nc.dram_tensor("out", (n_tok, DIM), F32, kind="ExternalOutput")

    with tile.TileContext(nc) as tc:
        with (
            tc.tile_pool(name="consts", bufs=1) as consts,
            tc.tile_pool(name="xtp", bufs=2) as xtp,
            tc.tile_pool(name="qkp", bufs=2) as qkp,
            tc.tile_pool(name="vp", bufs=2) as vp,
            tc.tile_pool(name="rowp", bufs=2) as rowp,
            tc.tile_pool(name="proj_ps", bufs=3, space="PSUM") as proj_ps,
            tc.tile_pool(name="att_ps", bufs=2, space="PSUM") as att_ps,
        ):
            # ---------------- tiny consts first ----------------
            ident = consts.tile([128, 128], F32)
            nc.sync.dma_start(ident[:], id_d[:])

            am_all = consts.tile([128, M], F32)
            if n_rows < 128:
                nc.vector.memset(am_all[:], 1.0)
            nc.sync.dma_start(am_all[0:n_rows, :], am_d[:])

            bq_sb = consts.tile([128, 8], F32)
            bk_sb = consts.tile([128, 8], F32)
            nc.sync.dma_start(bq_sb[:], bq_d[:].rearrange("(o p) -> p o", p=128))
            nc.sync.dma_start(bk_sb[:], bk_d[:].rearrange("(o p) -> p o", p=128))

            if use_bv:
                # bv as a K=1 bf16 pair for psum-accumulate
                ones_f = consts.tile([1, 128], F32)
                nc.vector.memset(ones_f[:], 1.0)
                ones_r = consts.tile([1, 128], BF16)
                nc.vector.tensor_copy(ones_r[:], ones_f[:])
                bv_row = consts.tile([1, DIM], F32)
                nc.sync.dma_start(bv_row[:],
                                  bv_d[:].rearrange("(a o) -> a o", a=1))
                bv_r = consts.tile([1, DIM], BF16)
                nc.vector.tensor_copy(bv_r[:], bv_row[:])

            # block-diagonal k stationaries: per (row, head-pair) a
            # [128, 128] block diag(k_even^T | k_odd^T). Zero blocks are
            # memset once and persist; the k-projection evacuation
            # rewrites only the diagonal blocks. Two buffers ping-pong
            # by tile parity.
            kd_bufs = []
            for i in range(2):
                kd = consts.tile([128, rows_per_tile * 8 * 128], BF16,
                                 name=f"kd{i}")
                nc.vector.memset(kd[:], 0.0)
                kd_bufs.append(kd)

            # block-diagonal e stationaries (one row each), 4 ping-pong
            ed_bufs = []
            for i in range(4):
                ed = consts.tile([128, 8 * 128], BF16, name=f"ed{i}")
                nc.vector.memset(ed[:], 0.0)
                ed_bufs.append(ed)

            # v-moving buffers: per head-pair hp a 65-col block
            # [128 = v_even | v_odd, 65 = hd | 1]; ones columns persist.
            v2bufs = []
            for i in range(4):
                v2b = consts.tile([128, 8 * 65], BF16, name=f"v2b{i}")
                nc.vector.memset(v2b[:], 1.0)
                v2bufs.append(v2b)

            # mask bias columns: mcolT2[:, r] = -1e4*(1 - am[r, j]) on both
            # partition halves (exp-bias per key token j)
            mcolT2 = consts.tile([128, 128], F32)
            amt_ps = att_ps.tile([128, 512], F32, tag="att")
            nc.tensor.transpose(amt_ps[0:M, 0:128], am_all[:], ident[:])
            nc.vector.tensor_scalar(
                mcolT2[0:64, :], amt_ps[0:M, 0:128], -NEG, NEG,
                op0=ALU.mult, op1=ALU.add)
            nc.vector.tensor_scalar(
                mcolT2[64:128, :], amt_ps[0:M, 0:128], -NEG, NEG,
                op0=ALU.mult, op1=ALU.add)

            # ---------------- weights + x tiles: direct bf16 DMA ---------
            # sync + scalar are HW DGE queues and carry all bulk traffic;
            # gpsimd (slow SW queue) keeps only memsets/sim/v-fills. DMA
            # issues on the scalar queue are safe only when they can never
            # block (buffers 3-deep), else they'd stall ACT evacuations.
            qhw = [nc.sync, nc.scalar]

            def emit_xt(ti):
                t0 = ti * T_TILE
                xt = [xtp.tile([128, T_TILE], BF16, tag=f"xt{d}", bufs=3,
                               name=f"xt{d}_{ti}") for d in range(8)]
                for dch in range(8):
                    nc.sync.dma_start(
                        xt[dch][:],
                        xt_d[128 * dch:128 * dch + 128, t0:t0 + T_TILE])
                return xt

            def emit_sim(ti):
                # all 8 rows' simT for the tile in 2 DMAs
                r0 = ti * rows_per_tile
                simt8 = rowp.tile([128, T_TILE], F32, tag="sim8",
                                  name=f"sim8_{ti}")
                src = sim_d[r0:r0 + rows_per_tile, :, :].rearrange(
                    "r j q -> j r q")
                for st in (0, 64):
                    nc.gpsimd.dma_start(
                        simt8[st:st + 64, :]
                        .rearrange("j (r q) -> j r q", q=M),
                        src)
                return simt8

            # tile-0 x chunks interleaved with wq chunks on the two HW
            # queues so the first q-projection matmuls can start after
            # ~2 transfers; wk/wv follow while tile-0 q-projections run.
            xt0 = [xtp.tile([128, T_TILE], BF16, tag=f"xt{d}", bufs=3,
                            name=f"xt{d}_0") for d in range(8)]
            wts = {name: [consts.tile([128, DIM], BF16, tag=f"w{name}{d}",
                                      name=f"w{name}{d}") for d in range(8)]
                   for name in ("q", "k", "v")}
            # first-group wq column slices (32 KB each) land before the
            # bulk so the och=0 projection can start as early as possible
            for dch in range(8):
                qhw[dch % 2].dma_start(
                    wts["q"][dch][:, 0:128],
                    wq_d[128 * dch:128 * dch + 128, 0:128])
            for dch in range(8):
                qhw[(dch + 1) % 2].dma_start(
                    xt0[dch][:], xt_d[128 * dch:128 * dch + 128, 0:T_TILE])
            for dch in range(8):
                qhw[dch % 2].dma_start(
                    wts["q"][dch][:, 128:DIM],
                    wq_d[128 * dch:128 * dch + 128, 128:DIM])
            for wi, (name, w_d) in enumerate(
                    (("k", wk_d), ("v", wv_d))):
                for dch in range(8):
                    qhw[(wi + dch) % 2].dma_start(
                        wts[name][dch][:],
                        w_d[128 * dch:128 * dch + 128, :])
            wqt, wkt, wvt = wts["q"], wts["k"], wts["v"]

            # ---------------- main loop over token tiles ----------------
            def make_proj(ti, xt):
                qt = [qkp.tile([128, T_TILE], BF16, tag=f"qt{o}",
                               name=f"qt{o}_{ti}") for o in range(8)]
                kd = kd_bufs[ti % 2]
                vts = [vp.tile([128, DIM], BF16, tag=f"v{s}",
                               name=f"v{s}_{ti}") for s in range(SUB)]
                groups = []

                def q_group(och):
                    ps = proj_ps.tile([128, T_TILE], F32, tag="proj",
                                      name=f"qkps{och}_{ti}")
                    for dch in range(8):
                        nc.tensor.matmul(
                            ps[:],
                            wqt[dch][:, 128 * och:128 * och + 128],
                            xt[dch][:],
                            start=(dch == 0), stop=(dch == 7),
                        )
                    nc.scalar.activation(
                        qt[och][:], ps[:], AF.Identity,
                        bias=bq_sb[:, och:och + 1], scale=1.0,
                    )

                def k_group(och):
                    # evacuate k straight into block-diagonal stationary
                    # form: kd[p<64, rr*1024 + och*128 + c] = k_even,
                    # kd[p>=64, ... + 64 + c] = k_odd; zero blocks persist
                    ps = proj_ps.tile([128, T_TILE], F32, tag="proj",
                                      name=f"kps{och}_{ti}")
                    for dch in range(8):
                        nc.tensor.matmul(
                            ps[:],
                            wkt[dch][:, 128 * och:128 * och + 128],
                            xt[dch][:],
                            start=(dch == 0), stop=(dch == 7),
                        )
                    kdv = kd[:].rearrange("p (rr blk) -> p rr blk", blk=1024)
                    for st in (0, 64):
                        nc.scalar.activation(
                            kdv[st:st + 64, :,
                                128 * och + st:128 * och + st + 64],
                            ps[st:st + 64, :]
                            .rearrange("p (rr c) -> p rr c", c=M),
                            AF.Identity,
                            bias=bk_sb[st:st + 64, och:och + 1], scale=1.0,
                        )

                def v_group(sub, oh):
                    vt = vts[sub]
                    ps = proj_ps.tile([128, 512], F32, tag="proj",
                                      name=f"vps{sub}{oh}_{ti}")
                    sl = slice(512 * oh, 512 * oh + 512)
                    for dch in range(8):
                        nc.tensor.matmul(
                            ps[:],
                            xt[dch][:, 128 * sub:128 * sub + 128],
                            wvt[dch][:, 512 * oh:512 * oh + 512],
                            start=(dch == 0), stop=(dch == 7) and not use_bv,
                        )
                    if use_bv:
                        nc.tensor.matmul(
                            ps[:], ones_r[:], bv_r[:, sl],
                            start=False, stop=True,
                        )
                    # DVE, not ACT: keeps the ACT FIFO short so q/k psum
                    # evacuations (which gate the PE) never queue behind it
                    nc.vector.tensor_copy(vt[:, sl], ps[:])

                for och in range(8):
                    groups.append(lambda och=och: q_group(och))
                for och in range(8):
                    groups.append(lambda och=och: k_group(och))
                for sub in range(SUB):
                    for oh in range(2):
                        groups.append(lambda sub=sub, oh=oh: v_group(sub, oh))
                return qt, kd, vts, groups

            def make_att_rows(ti, qt, kd, vts, simt8, la=1):
                rowstate = {}

                def att_row_a(rr):
                    r = ti * rows_per_tile + rr

                    # prefetch this row's v blocks one pipeline step early
                    rp = rr % 2
                    vt = vts[rr // 2]
                    v2b = v2bufs[rr % 4]
                    vsrc = (vt[64 * rp:64 * rp + 64, :]
                            .rearrange("p (b two c) -> p b two c",
                                       two=2, c=64))
                    v2bv = v2b[:].rearrange("p (b c) -> p b c", c=65)
                    for i, st in enumerate((0, 64)):
                        nc.sync.dma_start(
                            v2bv[st:st + 64, :, 0:64], vsrc[:, :, i, :])

                    # scores transposed: S'[j, q]; stationary = kd block
                    # (block-diagonal pair of heads), moving = qT slice
                    s_ps = att_ps.tile([128, 512], F32, tag="att",
                                       name=f"s_{r}")
                    tsl = slice(M * rr, M * rr + M)
                    for hp in range(8):
                        nc.tensor.matmul(
                            s_ps[:, 64 * hp:64 * hp + 64],
                            kd[:, 1024 * rr + 128 * hp:
                               1024 * rr + 128 * hp + 128],
                            qt[hp][:, tsl],
                            start=True, stop=True,
                        )

                    # t = S' * simT (DVE); ed = exp(t + maskcol) (ACT),
                    # written straight into block-diagonal stationary form,
                    # head-pairs 0-3 | 4-7 so ctx can start on the first
                    # half while the second is still on ACT
                    tt = rowp.tile([128, 512], F32, tag="tt", name=f"tt_{r}")
                    ed = ed_bufs[rr % 4]
                    edv = ed[:].rearrange("p (b c) -> p b c", c=128)
                    simr = simt8[:, M * rr:M * rr + M]
                    nc.vector.tensor_tensor(
                        tt[:].rearrange("p (a j) -> p a j", j=M),
                        s_ps[:].rearrange("p (a j) -> p a j", j=M),
                        simr.rearrange("p (a j) -> p a j", a=1)
                        .broadcast_to([128, 8, M]),
                        op=ALU.mult,
                    )
                    for st in (0, 64):
                        nc.scalar.activation(
                            edv[st:st + 64, :, st:st + 64],
                            tt[st:st + 64, :]
                            .rearrange("p (b c) -> p b c", c=M),
                            AF.Exp, bias=mcolT2[st:st + 64, r:r + 1])
                    rowstate[rr] = ed

                def att_row_b(rr):
                    r = ti * rows_per_tile + rr
                    ed = rowstate.pop(rr)
                    v2b = v2bufs[rr % 4]

                    # ctx' and denominators in one pass: stationary = ed
                    # block, moving = [v_h | 1] -> out[q, 0:64] = ctx',
                    # out[q, 64] = denom. Head-pairs 0-3 -> ctxa, 4-7 ->
                    # ctxb ([128,512] banks; only cols 0:260 used).
                    ctxa = att_ps.tile([128, 512], F32, tag="ctxa",
                                       name=f"ctxa_{r}")
                    ctxb = att_ps.tile([128, 512], F32, tag="ctxb",
                                       name=f"ctxb_{r}", bufs=1)
                    for hp in range(8):
                        dst = ctxa if hp < 4 else ctxb
                        col = 65 * (hp % 4)
                        nc.tensor.matmul(
                            dst[:, col:col + 65],
                            ed[:, 128 * hp:128 * hp + 128],
                            v2b[:, 65 * hp:65 * hp + 65],
                            start=True, stop=True,
                        )

                    # normalize while evacuating: out = ctx' * 1/denom
                    rcp = rowp.tile([128, 8], F32, tag="rcp", name=f"rcp_{r}")
                    osb = rowp.tile([128, 512], F32, tag="osb",
                                    name=f"osb_{r}")
                    for i, cx in enumerate((ctxa, ctxb)):
                        cxv = cx[:, 0:260].rearrange("p (b c) -> p b c", c=65)
                        nc.vector.reciprocal_approx_fast(
                            out=rcp[:, 4 * i:4 * i + 4]
                            .rearrange("p (b c) -> p b c", c=1),
                            in_=cxv[:, :, 64:65])
                        nc.vector.tensor_tensor(
                            osb[:, 256 * i:256 * i + 256]
                            .rearrange("p (b c) -> p b c", c=64),
                            cxv[:, :, 0:64],
                            rcp[:, 4 * i:4 * i + 4]
                            .rearrange("p (b o) -> p b o", o=1)
                            .broadcast_to([128, 4, 64]),
                            op=ALU.mult)

                    # out[64r + q, 64h + hd]; strip par holds heads 2hp+par
                    ov = out_d[M * r:M * r + M, :].rearrange(
                        "q (hp two hd) -> q hp two hd", two=2, hd=64)
                    for par in range(2):
                        nc.sync.dma_start(
                            ov[:, :, par, :],
                            osb[64 * par:64 * par + 64, :]
                            .rearrange("q (hp hd) -> q hp hd", hd=64),
                        )

                units = [lambda rr=rr: att_row_a(rr) for rr in range(la)]
                for rr in range(la, rows_per_tile):
                    units.append(lambda rr=rr: att_row_a(rr))
                    units.append(lambda rr=rr: att_row_b(rr - la))
                for rr in range(rows_per_tile - la, rows_per_tile):
                    units.append(lambda rr=rr: att_row_b(rr))
                return units

            prev_rows = []
            for ti in range(n_tiles):
                xt = xt0 if ti == 0 else emit_xt(ti)
                simt8 = emit_sim(ti)
                qt, kd, vts, groups = make_proj(ti, xt)
                ri = 0
                for gi, g in enumerate(groups):
                    g()
                    while (ri < len(prev_rows)
                           and (gi + 1) * len(prev_rows) // len(groups) > ri):
                        prev_rows[ri]()
                        ri += 1
                while ri < len(prev_rows):
                    prev_rows[ri]()
                    ri += 1
                prev_rows = make_att_rows(
                    ti, qt, kd, vts, simt8,
                    la=(3 if ti == n_tiles - 1 else 1))
            for row in prev_rows:
                row()

    return dict(out=out_d)


def _prepare_shards(hidden_states, attention_mask, sim_graph, Wq, bq, Wk, bk, Wv, bv,
                    n_cores=N_CORES):
    from ml_dtypes import bfloat16
    b, m, seq, dim = hidden_states.shape
    R = b * seq
    hs = np.transpose(np.asarray(hidden_states), (0, 2, 1, 3)).reshape(R, m, dim)
    am = np.ascontiguousarray(
        np.transpose(np.asarray(attention_mask), (0, 2, 1)).reshape(R, m),
        dtype=np.float32)
    sim = np.ascontiguousarray(
        np.transpose(np.asarray(sim_graph), (0, 2, 1)), dtype=np.float32)
    ident = np.eye(128, dtype=np.float32)
    WqT = np.ascontiguousarray((np.asarray(Wq).T * 0.125).astype(bfloat16))
    WkT = np.ascontiguousarray(np.asarray(Wk).T.astype(bfloat16))
    WvT = np.ascontiguousarray(np.asarray(Wv).T.astype(bfloat16))
    rows_per_core = R // n_cores
    in_maps = []
    for c in range(n_cores):
        r0 = c * rows_per_core
        xT = np.ascontiguousarray(
            hs[r0:r0 + rows_per_core].reshape(rows_per_core * m, dim)
            .T.astype(bfloat16))
        in_maps.append(dict(
            xT=xT,
            simg=sim[r0:r0 + rows_per_core],
            am=am[r0:r0 + rows_per_core],
            WqT=WqT, WkT=WkT, WvT=WvT,
            bq=np.ascontiguousarray(np.asarray(bq) * 0.125, np.float32),
            bk=np.ascontiguousarray(bk, np.float32),
            bv=np.ascontiguousarray(bv, np.float32),
            ident=ident,
        ))
    return in_maps


_CACHE = {}


def _get_compiled(use_bv=True):
    key = ("nc", use_bv)
    if key not in _CACHE:
        nc = bacc.Bacc("TRN2", target_bir_lowering=False, debug=False)
        build_core_kernel(nc, use_bv=use_bv)
        nc.compile()
        _CACHE[key] = nc
    return _CACHE[key]


LAST_EXEC_NS = [None]


def kernel(hidden_states, attention_mask, sim_graph, Wq, bq, Wk, bk, Wv, bv,
           b=4, m=64, seq=256, dim=1024, **_):
    from concourse import bass2jax

    use_bv = bool(np.any(np.asarray(bv)))
    nc = _get_compiled(use_bv=use_bv)
    in_maps = _prepare_shards(hidden_states, attention_mask, sim_graph,
                              Wq, bq, Wk, bk, Wv, bv)
    res = bass2jax.run_bass_via_pjrt(nc, in_maps, n_cores=N_CORES)
    R = int(b) * int(seq)
    out = np.concatenate([res[c]["out"] for c in range(N_CORES)], axis=0)
    return out.reshape(R, int(m), int(dim))
